# revision 1
# baseline (speedup 1.0000x reference)
"""CapsuleLayer dynamic-routing kernel for 8 Trainium2 NeuronCores.

Problem: x [128, 2048, 8], W [32, 2048, 16, 8] (fp32)
  u_hat[b,j,i,d] = sum_p W[j,i,d,p] * x[b,i,p]
  3 rounds of routing-by-agreement (softmax over j, squash), no
  persistent logits needed: b_k = (sum_{m<k} out_m) . u_hat, so each
  round is a streaming pass over i needing only O_k = sum out_m.

Sharding: i (input capsules) split 8 ways; every core holds the full
batch B=128 on SBUF partitions. Per-round partial sums s[b,(j,d)]
(256KB/core) are reduced on the host between the three launches.
"""

import numpy as np
from contextlib import ExitStack

import concourse.bass as bass
import concourse.mybir as mybir
from concourse import tile
from concourse.bass_utils import run_bass_kernel_spmd

# ---------------------------------------------------------------------------
# Shapes (hardcoded for this problem)
B, I, P = 128, 2048, 8
J, D = 32, 16
JD = J * D               # 512
N_CORES = 8
I_LOC = I // N_CORES     # 256
EPS = 1e-7
GROUP = 4                # i's per routing group (psum tile = GROUP banks)
N_GROUPS = I_LOC // GROUP

_f32 = mybir.dt.float32


# ---------------------------------------------------------------------------
# Walrus compat: this toolchain rejects sync waits on InstDrain and >2 on
# InstEventSemaphore. Emit the waits as standalone nops before the drain.
def _apply_tile_compat():
    from concourse.vector_clock import ScopedClock

    def _strip_waits(inst):
        si = inst.sync_info
        if not si or not si.on_wait:
            return []
        waits = list(si.on_wait)
        si.on_wait = []
        inst.sync_info = si
        return waits

    def _nop_with_wait(eng, w):
        nop = eng.nop(nofuse=True, hint="drain_wait_split")
        nsi = nop.ins.sync_info
        if nsi is None:
            nsi = mybir.SyncInfo(on_wait=[], on_update=[])
        nsi.on_wait = list(nsi.on_wait or []) + [w]
        nop.ins.sync_info = nsi

    def _patched_multi_engine_barrier(self, engines):
        for inst in bass._bass_rust._multi_engine_barrier_insts(
            self, list(engines)
        ):
            eng = self.engines[inst.engine]
            for w in _strip_waits(inst):
                _nop_with_wait(eng, w)
            eng.add_instruction(inst)

    def _patched_drain_and_barrier(self, tick_clock, wait_clock):
        nop_inst = self.nc.sync.nop(nofuse=True, hint="drain_wait_split")
        wait_clock.add_sem_waits(
            nop_inst.ins, ScopedClock({None: tick_clock.global_clock})
        )
        si = nop_inst.ins.sync_info
        if si and si.on_wait and len(si.on_wait) > 1:
            extra = list(si.on_wait[1:])
            si.on_wait = [si.on_wait[0]]
            nop_inst.ins.sync_info = si
            for w in extra:
                _nop_with_wait(self.nc.sync, w)
        self.nc.sync.drain()

        self.nc.all_engine_barrier()
        assert self.sems is not None
        popped = self.nc._tile_sem_poison_stack.pop()
        assert popped is self._sem_poison
        self.nc.clear_and_free_semaphores(list(self.sems.allocated().values()))
        # No trailing all_engine_barrier: every engine is already past the
        # pre-clear barrier (done touching semaphores), nothing reads them
        # afterwards, and NEFF completion only needs each engine to halt.

    # Scheduled body instructions can also end up with >1 wait (e.g. a
    # matmul waiting on two DMAs). Spill extras onto same-engine NoOps
    # inserted immediately before the instruction.
    _WAIT_CAPS = {"InstDrain": 0, "InstEventSemaphore": 2}
    _orig_add_instruction = tile.TileContext._add_instruction

    def _patched_add_instruction(self, inst):
        si = inst.sync_info
        cap = _WAIT_CAPS.get(type(inst).__name__, 1)
        if si and si.on_wait and len(si.on_wait) > cap:
            waits = list(si.on_wait)
            si.on_wait = waits[:cap]
            inst.sync_info = si
            for w in waits[cap:]:
                nop = mybir.InstNoOp(
                    name=f"I-{self.nc.next_id()}-waitspill", ins=[], outs=[]
                )
                nop.engine = inst.engine
                nop.sync_info = mybir.SyncInfo(on_wait=[w], on_update=[])
                _orig_add_instruction(self, nop)
        _orig_add_instruction(self, inst)

    bass.Bass.multi_engine_barrier = _patched_multi_engine_barrier
    tile.TileContext._drain_and_barrier = _patched_drain_and_barrier
    tile.TileContext._add_instruction = _patched_add_instruction


_apply_tile_compat()


# ---------------------------------------------------------------------------
# Launch 1: s0_part[b,(j,d)] = sum_{i local} u_hat[b,j,i,d]
# (iteration 0 has exactly uniform c = 1/32, applied on the host)
def build_l1():
    nc = bass.Bass("TRN2", target_bir_lowering=False, debug=False)
    n_chunks = (I_LOC * P) // 128  # 16
    xw1 = nc.dram_tensor(
        "xw1", [n_chunks, 128, B + JD], _f32, kind="ExternalInput").ap()
    sp = nc.dram_tensor("sp", [B, JD], _f32, kind="ExternalOutput").ap()
    with ExitStack() as ctx:
        tc = ctx.enter_context(tile.TileContext(nc))
        xpool = ctx.enter_context(tc.tile_pool(name="xw1", bufs=4))
        ppool = ctx.enter_context(tc.tile_pool(name="ps", bufs=1, space="PSUM"))
        opool = ctx.enter_context(tc.tile_pool(name="o", bufs=1))
        psum = ppool.tile([B, JD], _f32)
        for q in range(n_chunks):
            t = xpool.tile([128, B + JD], _f32)
            nc.sync.dma_start(t[:], xw1[q])
            nc.tensor.matmul(
                psum[:], lhsT=t[:, :B], rhs=t[:, B:],
                start=(q == 0), stop=(q == n_chunks - 1),
            )
        out = opool.tile([B, JD], _f32)
        nc.scalar.copy(out[:], psum[:])
        nc.sync.dma_start(sp[:], out[:])
    return nc


# ---------------------------------------------------------------------------
# Launches 2 & 3: one routing round.
#   g[b,j,i]  = sum_d O[b,j,d] * u_hat[b,j,i,d]
#   c         = softmax_j(g)
#   s_part    = sum_{i local} c * u_hat
#
# x and W arrive interleaved in 16-i blocks ("xw": per block, the x
# slab [P, 16*B] then the W slab [P, 16*JD], both p-major) so each
# block is one large DMA instead of 16 small ones.
BLK = 16                       # i's per DMA block
N_BLKS = I_LOC // BLK          # 16
XW_X = BLK * B                 # 2048 x columns per block
XW_W = BLK * JD                # 8192 W columns per block
XW_COLS = XW_X + XW_W          # 10240


def build_l2():
    nc = bass.Bass("TRN2", target_bir_lowering=False, debug=False)
    xw = nc.dram_tensor(
        "xw", [N_BLKS, P, XW_COLS], _f32, kind="ExternalInput").ap()
    x2d = nc.dram_tensor("x2", [B, I_LOC * P], _f32, kind="ExternalInput").ap()
    w2d = nc.dram_tensor(
        "w2", [J, D, I_LOC * P], _f32, kind="ExternalInput").ap()
    otd = nc.dram_tensor("ot", [D, J * B], _f32, kind="ExternalInput").ap()
    sp = nc.dram_tensor("sp", [B, JD], _f32, kind="ExternalOutput").ap()

    IP = I_LOC * P  # 2048

    with ExitStack() as ctx:
        tc = ctx.enter_context(tile.TileContext(nc))
        wpool = ctx.enter_context(tc.tile_pool(name="xw", bufs=2))
        tpool = ctx.enter_context(tc.tile_pool(name="tmp", bufs=2))
        gpool = ctx.enter_context(tc.tile_pool(name="g", bufs=2))
        bpool = ctx.enter_context(tc.tile_pool(name="big", bufs=1))
        apool = ctx.enter_context(tc.tile_pool(name="acc", bufs=1))

        # wide accumulators: one GROUP-lane per i-position, reduced once
        # at the end. Two of them so even groups accumulate on DVE and
        # odd groups on GpSimd, halving the DVE add chain.
        s_wide = apool.tile([B, GROUP * JD], _f32)
        nc.gpsimd.memset(s_wide[:], 0.0)
        s_wide2 = apool.tile([B, GROUP * JD], _f32)
        nc.gpsimd.memset(s_wide2[:], 0.0)

        # ---- phase A: g[b,(j,i)] = sum_p x2[b,(i,p)] * (O_j @ W2_j)[b,(i,p)]
        # g_all is reused in place for e = exp(g - m) and then c (softmax
        # numerator / weights): every op is elementwise with identical
        # input/output traversal order.
        g_all = bpool.tile([B, J * I_LOC], _f32)

        with tc.tile_pool(name="vps", bufs=2, space="PSUM") as vppool, \
             tc.tile_pool(name="pa", bufs=1) as papool, \
             tc.tile_pool(name="w2", bufs=2) as w2pool:
            x2 = papool.tile([B, IP], _f32)
            nc.sync.dma_start(x2[:], x2d[:])
            ot = papool.tile([D, J * B], _f32)
            nc.sync.dma_start(ot[:], otd[:])
            for j in range(J):
                w2t = w2pool.tile([D, IP], _f32)
                nc.sync.dma_start(w2t[:], w2d[j])
                vps = vppool.tile([B, IP], _f32)
                for q in range(IP // 512):
                    nc.tensor.matmul(
                        vps[:, q * 512:(q + 1) * 512],
                        lhsT=ot[:, j * B:(j + 1) * B],
                        rhs=w2t[:, q * 512:(q + 1) * 512],
                        start=True, stop=True,
                    )
                xv = tpool.tile([B, IP], _f32)
                nc.vector.tensor_tensor(
                    xv[:], x2[:], vps[:], op=mybir.AluOpType.mult,
                )
                nc.vector.reduce_sum(
                    g_all[:, j * I_LOC:(j + 1) * I_LOC],
                    xv[:].rearrange("b (i p) -> b i p", i=I_LOC, p=P),
                    axis=mybir.AxisListType.X,
                )

        # ---- softmax over j (free-dim strided, one shot for all i).
        # No max-subtraction: g = O.u_hat with squashed O (|O_j| < 1) is
        # bounded well inside exp's fp32 range, and softmax is shift-
        # invariant, so exp(g)/sum exp(g) matches the reference exactly.
        gjv = g_all[:].rearrange("b (j i) -> b j i", j=J, i=I_LOC)
        giv = g_all[:].rearrange("b (j i) -> b i j", j=J, i=I_LOC)
        nc.scalar.activation(
            g_all[:], g_all[:], mybir.ActivationFunctionType.Exp
        )
        Z = bpool.tile([B, I_LOC], _f32)
        nc.vector.reduce_sum(Z[:], giv, axis=mybir.AxisListType.X)
        Zr = bpool.tile([B, I_LOC], _f32)
        nc.vector.reciprocal(Zr[:], Z[:])
        nc.vector.tensor_tensor(
            gjv, gjv, Zr[:].unsqueeze(1).broadcast_to([B, J, I_LOC]),
            op=mybir.AluOpType.mult,
        )
        c_v = giv

        # ---- phase B: s += sum_i c * u_hat, u_hat recomputed per group.
        # The weighted tiles w are accumulated on the PE into a resident
        # PSUM region via identity matmuls (start=False), so the DVE only
        # does the c-multiply.
        ppool = ctx.enter_context(tc.tile_pool(name="ps", bufs=2, space="PSUM"))
        xw_tiles = {}
        for gi in range(N_GROUPS):
            blk, sub = divmod(gi * GROUP, BLK)
            if sub == 0:
                xwt = wpool.tile([P, XW_COLS], _f32)
                nc.sync.dma_start(xwt[:], xw[blk])
                xw_tiles[blk] = xwt
            xwt = xw_tiles[blk]
            psum = ppool.tile([B, GROUP * JD], _f32)
            for t in range(GROUP):
                ib = sub + t           # i index within the block
                nc.tensor.matmul(
                    psum[:, t * JD:(t + 1) * JD],
                    lhsT=xwt[:, ib * B:(ib + 1) * B],
                    rhs=xwt[:, XW_X + ib * JD:XW_X + (ib + 1) * JD],
                    start=True, stop=True,
                )
            pv = psum[:].rearrange("b (i j d) -> b i j d", i=GROUP, j=J, d=D)
            cslice = c_v[:, gi * GROUP:(gi + 1) * GROUP, :]
            w = tpool.tile([B, GROUP * JD], _f32)
            wv = w[:].rearrange("b (i j d) -> b i j d", i=GROUP, j=J, d=D)
            nc.vector.tensor_tensor(
                wv, pv, cslice.unsqueeze(3).broadcast_to([B, GROUP, J, D]),
                op=mybir.AluOpType.mult,
            )
            if gi % 2 == 0:
                nc.vector.tensor_add(s_wide[:], s_wide[:], w[:])
            else:
                nc.gpsimd.tensor_add(s_wide2[:], s_wide2[:], w[:])

        nc.vector.tensor_add(s_wide[:], s_wide[:], s_wide2[:])
        s_acc = gpool.tile([B, JD], _f32)
        nc.vector.reduce_sum(
            s_acc[:],
            s_wide[:].rearrange("b (i jd) -> b jd i", i=GROUP, jd=JD),
            axis=mybir.AxisListType.X,
        )
        nc.sync.dma_start(sp[:], s_acc[:])
    return nc


# ---------------------------------------------------------------------------
# Host glue
def _squash(s):
    v = s.reshape(B, J, D)
    s2 = np.sum(np.square(v), axis=-1, keepdims=True)
    scale = s2 / (1.0 + s2) / np.sqrt(s2 + EPS)
    return (scale * v).astype(np.float32)


_cache = {}


def _get_nc(name):
    if name not in _cache:
        _cache[name] = build_l1() if name == "l1" else build_l2()
    return _cache[name]


def _prep_inputs(x, W):
    """Per-core host-side re-layouts (all fp32, cheap transposes)."""
    per_core = []
    for c in range(N_CORES):
        sl = slice(c * I_LOC, (c + 1) * I_LOC)
        xc = x[:, sl, :]                                   # [B, I_LOC, P]
        wc = W[:, sl, :, :]                                # [J, I_LOC, D, P]
        xp = np.ascontiguousarray(
            xc.transpose(1, 2, 0).reshape(I_LOC * P, B))   # (i,p),b
        wt = np.ascontiguousarray(
            wc.transpose(1, 3, 0, 2).reshape(I_LOC * P, JD))  # (i,p),(j,d)
        # L1: interleave x/W per 128-row chunk so each chunk is one DMA
        n_chunks = (I_LOC * P) // 128
        xw1 = np.empty((n_chunks, 128, B + JD), np.float32)
        xw1[:, :, :B] = xp.reshape(n_chunks, 128, B)
        xw1[:, :, B:] = wt.reshape(n_chunks, 128, JD)
        # interleaved blocks for L2/L3: per 16-i block, [P, 16*B | 16*JD]
        xb = xc.transpose(2, 1, 0).reshape(P, N_BLKS, BLK * B)  # p,(blk,i*b)
        wb = wc.transpose(3, 1, 0, 2).reshape(P, N_BLKS, BLK, JD)
        xw = np.empty((N_BLKS, P, XW_COLS), np.float32)
        xw[:, :, :XW_X] = xb.transpose(1, 0, 2)
        xw[:, :, XW_X:] = wb.transpose(1, 0, 2, 3).reshape(N_BLKS, P, XW_W)
        # V-trick layouts
        x2 = np.ascontiguousarray(xc.reshape(B, I_LOC * P))      # b,(i,p)
        w2 = np.ascontiguousarray(
            wc.transpose(0, 2, 1, 3).reshape(J, D, I_LOC * P))   # j,d,(i,p)
        per_core.append({"xw1": xw1, "xw": xw, "x2": x2, "w2": w2})
    return per_core


def _ot_layout(O):
    """O [B, JD] -> lhsT layout [D, J*B] for the V matmuls."""
    return np.ascontiguousarray(
        O.reshape(B, J, D).transpose(2, 1, 0).reshape(D, J * B))


def _run(nc, in_maps, **kw):
    res = run_bass_kernel_spmd(nc, in_maps, list(range(N_CORES)), **kw)
    return res


def kernel(x, W, _collect_times=None):
    x = np.asarray(x, dtype=np.float32)
    W = np.asarray(W, dtype=np.float32)
    pc = _prep_inputs(x, W)

    nc1 = _get_nc("l1")
    nc2 = _get_nc("l2")

    r1 = _run(nc1, [{"xw1": p["xw1"]} for p in pc])
    s0 = np.sum([r1.results[c]["sp"] for c in range(N_CORES)], axis=0)
    s0 *= (1.0 / J)
    out0 = _squash(s0)
    O1 = out0.reshape(B, JD)

    ot1 = _ot_layout(O1)
    r2 = _run(nc2, [
        {"xw": p["xw"], "x2": p["x2"], "w2": p["w2"], "ot": ot1}
        for p in pc
    ])
    s1 = np.sum([r2.results[c]["sp"] for c in range(N_CORES)], axis=0)
    out1 = _squash(s1)
    O2 = (out0 + out1).reshape(B, JD)

    ot2 = _ot_layout(O2)
    r3 = _run(nc2, [
        {"xw": p["xw"], "x2": p["x2"], "w2": p["w2"], "ot": ot2}
        for p in pc
    ])
    s2 = np.sum([r3.results[c]["sp"] for c in range(N_CORES)], axis=0)
    out2 = _squash(s2)

    if _collect_times is not None:
        for r in (r1, r2, r3):
            _collect_times.append(r.exec_time_ns)
    return out2



# revision 4
# speedup vs baseline: 2.4178x; 2.4178x over previous
"""CapsuleLayer dynamic-routing kernel for 8 Trainium2 NeuronCores (v2).

Problem: x [128, 2048, 8], W [32, 2048, 16, 8] (fp32)
  u_hat[b,j,i,d] = sum_p W[j,i,d,p] * x[b,i,p]
  3 rounds of routing (softmax over j, squash). Logits never materialize:
  round-k coupling g = O_k . u_hat with O_k = sum of previous outputs.

Sharding: i (input capsules) split 8 ways (I_LOC=256/core); every core
holds the full batch. Host reduces the per-core partial sums s[b,j,d]
and applies squash between the three launches.

All device data is bf16 (tolerance 2e-2; bf16 lands ~1e-3):
 - fp32 matmuls cost 4 cyc/row on the PE vs 1 for bf16,
 - DVE gets the 2x perf mode only for 2-byte dtypes.

Round structure per launch (L2 = L3):
  phase A:  V_j[b,(i,p)] = O_j @ W_j           (PE, psum)
            Vb_j = bf16 copy of psum           (Act)
            xV_j = x * Vb_j                    (DVE, 2x)
            g_j[b,i] = sum_p xV_j              (bf16 add tree: lvl1 mostly
                                                on Pool, lvl2/3 on DVE)
            e = exp(g)                         (Act)
            eT[(i),(j,c,b)] via DMA-transpose  (DMA xbar, idle engine)
  softmax:  Z = sum_j e (add tree), Zr = 1/Z, x'T = xT * ZrT
  phase B:  xcT[(i),(j,p,b)] = eT * x'T        (DVE, 2x, stride-0 bcasts)
            sT[(jr,d),(jq,b)] += W_B^T @ xcT   (PE, 512 accumulating
                                                matmuls into one psum bank)
"""

import numpy as np
import ml_dtypes
from contextlib import ExitStack

import concourse.bass as bass
import concourse.mybir as mybir
from concourse import tile
from concourse.bass_utils import run_bass_kernel_spmd

# ---------------------------------------------------------------------------
B, I, P = 128, 2048, 8
J, D = 32, 16
JD = J * D               # 512
N_CORES = 8
I_LOC = I // N_CORES     # 256
N_CH = I_LOC // 128      # 2 partition chunks of i
EPS = 1e-7

_f32 = mybir.dt.float32
_bf16 = mybir.dt.bfloat16
_npbf = ml_dtypes.bfloat16

# engine split tuning
POOL_LVL1 = set(range(8, 32))     # j's whose tree lvl1 runs on Pool
ACT_COPY = set(range(32))         # j's whose psum->bf16 copy runs on Act


# ---------------------------------------------------------------------------
# Walrus compat: this toolchain rejects sync waits on InstDrain and >2 on
# InstEventSemaphore. Emit the waits as standalone nops before the drain.
def _apply_tile_compat():
    from concourse.vector_clock import ScopedClock

    def _strip_waits(inst):
        si = inst.sync_info
        if not si or not si.on_wait:
            return []
        waits = list(si.on_wait)
        si.on_wait = []
        inst.sync_info = si
        return waits

    def _nop_with_wait(eng, w):
        nop = eng.nop(nofuse=True, hint="drain_wait_split")
        nsi = nop.ins.sync_info
        if nsi is None:
            nsi = mybir.SyncInfo(on_wait=[], on_update=[])
        nsi.on_wait = list(nsi.on_wait or []) + [w]
        nop.ins.sync_info = nsi

    def _patched_multi_engine_barrier(self, engines):
        for inst in bass._bass_rust._multi_engine_barrier_insts(
            self, list(engines)
        ):
            eng = self.engines[inst.engine]
            for w in _strip_waits(inst):
                _nop_with_wait(eng, w)
            eng.add_instruction(inst)

    def _patched_drain_and_barrier(self, tick_clock, wait_clock):
        nop_inst = self.nc.sync.nop(nofuse=True, hint="drain_wait_split")
        wait_clock.add_sem_waits(
            nop_inst.ins, ScopedClock({None: tick_clock.global_clock})
        )
        si = nop_inst.ins.sync_info
        if si and si.on_wait and len(si.on_wait) > 1:
            extra = list(si.on_wait[1:])
            si.on_wait = [si.on_wait[0]]
            nop_inst.ins.sync_info = si
            for w in extra:
                _nop_with_wait(self.nc.sync, w)
        self.nc.sync.drain()

        self.nc.all_engine_barrier()
        assert self.sems is not None
        popped = self.nc._tile_sem_poison_stack.pop()
        assert popped is self._sem_poison
        self.nc.clear_and_free_semaphores(list(self.sems.allocated().values()))

    _WAIT_CAPS = {"InstDrain": 0, "InstEventSemaphore": 2}
    _orig_add_instruction = tile.TileContext._add_instruction

    def _patched_add_instruction(self, inst):
        si = inst.sync_info
        cap = _WAIT_CAPS.get(type(inst).__name__, 1)
        if si and si.on_wait and len(si.on_wait) > cap:
            waits = list(si.on_wait)
            si.on_wait = waits[:cap]
            inst.sync_info = si
            for w in waits[cap:]:
                nop = mybir.InstNoOp(
                    name=f"I-{self.nc.next_id()}-waitspill", ins=[], outs=[]
                )
                nop.engine = inst.engine
                nop.sync_info = mybir.SyncInfo(on_wait=[w], on_update=[])
                _orig_add_instruction(self, nop)
        _orig_add_instruction(self, inst)

    bass.Bass.multi_engine_barrier = _patched_multi_engine_barrier
    tile.TileContext._drain_and_barrier = _patched_drain_and_barrier
    tile.TileContext._add_instruction = _patched_add_instruction


_apply_tile_compat()

DEBUG_DUMPS = False

_MUL = mybir.AluOpType.mult
_ADD = mybir.AluOpType.add
_AX = mybir.AxisListType.X


# ---------------------------------------------------------------------------
# Launch 1: s0_part[b,(j,d)] = sum_{i local} u_hat[b,j,i,d]
# (iteration 0 has exactly uniform c = 1/32, applied on the host)
def build_l1():
    nc = bass.Bass("TRN2", target_bir_lowering=False, debug=False)
    n_chunks = (I_LOC * P) // 128  # 16
    xw1 = nc.dram_tensor(
        "xw1", [n_chunks, 128, B + JD], _bf16, kind="ExternalInput").ap()
    sp = nc.dram_tensor("sp", [B, JD], _f32, kind="ExternalOutput").ap()
    with ExitStack() as ctx:
        tc = ctx.enter_context(tile.TileContext(nc))
        xpool = ctx.enter_context(tc.tile_pool(name="xw1", bufs=4))
        ppool = ctx.enter_context(tc.tile_pool(name="ps", bufs=1, space="PSUM"))
        opool = ctx.enter_context(tc.tile_pool(name="o", bufs=1))
        psum = ppool.tile([B, JD], _f32)
        for q in range(n_chunks):
            t = xpool.tile([128, B + JD], _bf16)
            nc.sync.dma_start(t[:], xw1[q])
            nc.tensor.matmul(
                psum[:], lhsT=t[:, :B], rhs=t[:, B:],
                start=(q == 0), stop=(q == n_chunks - 1),
            )
        out = opool.tile([B, JD], _f32)
        nc.scalar.copy(out[:], psum[:])
        nc.sync.dma_start(sp[:], out[:])
    return nc


# ---------------------------------------------------------------------------
# Launches 2 & 3: one routing round.
def build_l2():
    nc = bass.Bass("TRN2", target_bir_lowering=False, debug=False)
    wa = nc.dram_tensor("wa", [128, 8 * I_LOC * P], _bf16,
                        kind="ExternalInput").ap()
    oa = nc.dram_tensor("oa", [128, 8 * B], _bf16, kind="ExternalInput").ap()
    xa = nc.dram_tensor("xa", [B, I_LOC * P], _bf16,
                        kind="ExternalInput").ap()
    xbt = nc.dram_tensor("xbt", [128, N_CH * P * B], _bf16,
                         kind="ExternalInput").ap()
    wb = nc.dram_tensor("wb", [128, J * N_CH * P * D], _bf16,
                        kind="ExternalInput").ap()
    sp = nc.dram_tensor("sp", [128, 1024], _f32, kind="ExternalOutput").ap()
    if DEBUG_DUMPS:
        dbg_e = nc.dram_tensor("dbg_e", [B, J * I_LOC], _bf16,
                               kind="ExternalOutput").ap()
        dbg_eT = nc.dram_tensor("dbg_eT", [128, J * N_CH * B], _bf16,
                                kind="ExternalOutput").ap()
        dbg_xpt = nc.dram_tensor("dbg_xpt", [128, N_CH * P * B], _bf16,
                                 kind="ExternalOutput").ap()
        dbg_z = nc.dram_tensor("dbg_z", [B, I_LOC], _f32,
                               kind="ExternalOutput").ap()

    IP = I_LOC * P  # 2048

    with ExitStack() as ctx:
        tc = ctx.enter_context(tile.TileContext(nc))
        # persistent inputs
        cpool = ctx.enter_context(tc.tile_pool(name="cst", bufs=1))
        t_oa = cpool.tile([128, 8 * B], _bf16)
        nc.sync.dma_start(t_oa[:], oa[:])
        t_xa = cpool.tile([B, IP], _bf16)
        nc.sync.dma_start(t_xa[:], xa[:])
        t_wa = cpool.tile([128, 8 * IP], _bf16)
        nc.sync.dma_start(t_wa[:], wa[:])
        t_xbt = cpool.tile([128, N_CH * P * B], _bf16)
        nc.sync.dma_start(t_xbt[:], xbt[:])
        t_wb = cpool.tile([128, J * N_CH * P * D], _bf16)
        nc.sync.dma_start(t_wb[:], wb[:])

        # e (= g, exp'd in place) and its transpose
        bpool = ctx.enter_context(tc.tile_pool(name="big", bufs=1))
        e_all = bpool.tile([B, J * I_LOC], _bf16)
        eT = bpool.tile([128, J * N_CH * B], _bf16)

        # ---- phase A ------------------------------------------------------
        with tc.tile_pool(name="vps", bufs=2, space="PSUM") as vppool, \
             tc.tile_pool(name="vb", bufs=2) as vbpool, \
             tc.tile_pool(name="xv", bufs=2) as xvpool, \
             tc.tile_pool(name="t4", bufs=2) as t4pool, \
             tc.tile_pool(name="t2", bufs=2) as t2pool:
            for j in range(J):
                grp, slot = divmod(j, 4)
                r0 = slot * 32
                lT = t_oa[r0:r0 + 32, grp * B:(grp + 1) * B]
                vps = vppool.tile([B, IP], _f32)
                for q in range(IP // 512):
                    nc.tensor.matmul(
                        vps[:, q * 512:(q + 1) * 512],
                        lhsT=lT,
                        rhs=t_wa[r0:r0 + 32,
                                 grp * IP + q * 512:grp * IP + (q + 1) * 512],
                        start=True, stop=True, tile_position=(r0, 0),
                    )
                xv = xvpool.tile([B, IP], _bf16)
                if j in ACT_COPY:
                    vb = vbpool.tile([B, IP], _bf16)
                    nc.scalar.copy(vb[:], vps[:])
                    nc.vector.tensor_tensor(xv[:], t_xa[:], vb[:], op=_MUL)
                else:
                    nc.vector.tensor_tensor(xv[:], t_xa[:], vps[:], op=_MUL)
                # p-reduction tree (views [b, i, p])
                xvv = xv[:].rearrange("b (i p) -> b i p", i=I_LOC, p=P)
                t4 = t4pool.tile([B, I_LOC * 4], _bf16)
                t4v = t4[:].rearrange("b (i p) -> b i p", i=I_LOC, p=4)
                eng1 = nc.gpsimd if j in POOL_LVL1 else nc.vector
                eng1.tensor_tensor(t4v, xvv[:, :, 0:4], xvv[:, :, 4:8],
                                   op=_ADD)
                t2 = t2pool.tile([B, I_LOC * 2], _bf16)
                t2v = t2[:].rearrange("b (i p) -> b i p", i=I_LOC, p=2)
                nc.vector.tensor_tensor(t2v, t4v[:, :, 0:2], t4v[:, :, 2:4],
                                        op=_ADD)
                gv = e_all[:, j * I_LOC:(j + 1) * I_LOC].rearrange(
                    "b (i u) -> b i u", i=I_LOC, u=1)
                nc.vector.tensor_tensor(gv, t2v[:, :, 0:1], t2v[:, :, 1:2],
                                        op=_ADD)
                if j % 2 == 1:
                    sl = e_all[:, (j - 1) * I_LOC:(j + 1) * I_LOC]
                    nc.scalar.activation(
                        sl, sl, mybir.ActivationFunctionType.Exp)
                    # eT[p=ilo, blk=(jloc, ihi), b] = e[b, blk*128 + ilo]
                    dst = eT[:, (j - 1) * N_CH * B:(j + 1) * N_CH * B]
                    nc.sync.dma_start_transpose(
                        dst.rearrange("p (g b) -> p g b", g=2 * N_CH, b=B),
                        sl)

        # ---- softmax normalizer ------------------------------------------
        spool = ctx.enter_context(tc.tile_pool(name="sm", bufs=1))
        zt1 = spool.tile([B, 4096], _bf16)
        nc.gpsimd.tensor_tensor(zt1[:], e_all[:, :4096], e_all[:, 4096:],
                                op=_ADD)
        zt2 = spool.tile([B, 2048], _bf16)
        nc.vector.tensor_tensor(zt2[:], zt1[:, :2048], zt1[:, 2048:], op=_ADD)
        zt3 = spool.tile([B, 1024], _bf16)
        nc.vector.tensor_tensor(zt3[:], zt2[:, :1024], zt2[:, 1024:], op=_ADD)
        zt4 = spool.tile([B, 512], _bf16)
        nc.vector.tensor_tensor(zt4[:], zt3[:, :512], zt3[:, 512:], op=_ADD)
        z = spool.tile([B, I_LOC], _f32)
        nc.vector.tensor_tensor(z[:], zt4[:, :256], zt4[:, 256:], op=_ADD)
        zr = spool.tile([B, I_LOC], _f32)
        nc.vector.reciprocal(zr[:], z[:])
        zrb = spool.tile([B, I_LOC], _bf16)
        nc.vector.tensor_copy(zrb[:], zr[:])
        zrT = spool.tile([128, N_CH * B], _bf16)
        nc.sync.dma_start_transpose(
            zrT[:].rearrange("p (g b) -> p g b", g=N_CH, b=B), zrb[:])
        # x'T[(ilo), (ihi, p, b)] = xT * (1/Z) broadcast over p
        xpt = spool.tile([128, N_CH * P * B], _bf16)
        nc.vector.tensor_tensor(
            xpt[:].rearrange("i (c p b) -> i c p b", c=N_CH, p=P, b=B),
            t_xbt[:].rearrange("i (c p b) -> i c p b", c=N_CH, p=P, b=B),
            zrT[:].rearrange("i (c b) -> i c b", c=N_CH, b=B)
                .unsqueeze(2).broadcast_to([128, N_CH, P, B]),
            op=_MUL)

        if DEBUG_DUMPS:
            nc.sync.dma_start(dbg_e[:], e_all[:])
            nc.sync.dma_start(dbg_eT[:], eT[:])
            nc.sync.dma_start(dbg_xpt[:], xpt[:])
            nc.sync.dma_start(dbg_z[:], z[:])

        # ---- phase B ------------------------------------------------------
        eTv = eT[:].rearrange("i (j c b) -> i j c b", j=J, c=N_CH, b=B)
        xptv = xpt[:].rearrange("i (c p b) -> i c p b", c=N_CH, p=P, b=B)
        ppool = ctx.enter_context(tc.tile_pool(name="psB", bufs=1,
                                               space="PSUM"))
        xcpool = ctx.enter_context(tc.tile_pool(name="xc", bufs=2))
        # one accumulation region per (partition-group x psum bank):
        # region j = rows (j%4)*32..+16, cols (j//4)*512..+128, so no two
        # regions share a (partition, bank) pair (start_tensor_calc zeroes
        # the full 2KB bank row on written partitions).
        psB = ppool.tile([128, 4096], _f32)
        for jg in range(4):
            for ci in range(N_CH):
                xcb = xcpool.tile([128, 8 * P * B], _bf16)
                xcv = xcb[:].rearrange("i (j p b) -> i j p b", j=8, p=P, b=B)
                nc.vector.tensor_tensor(
                    xcv,
                    eTv[:, jg * 8:(jg + 1) * 8, ci, :]
                        .unsqueeze(2).broadcast_to([128, 8, P, B]),
                    xptv[:, ci].unsqueeze(1).broadcast_to([128, 8, P, B]),
                    op=_MUL)
                for j8 in range(8):
                    j = jg * 8 + j8
                    sgrp, sslot = divmod(j, 4)
                    nc0 = j * (N_CH * P * D) + ci * (P * D)
                    for p in range(P):
                        nc.tensor.matmul(
                            psB[sslot * 32:sslot * 32 + 16,
                                sgrp * 512:sgrp * 512 + B],
                            lhsT=t_wb[:, nc0 + p * D:nc0 + (p + 1) * D],
                            rhs=xcv[:, j8, p, :],
                            start=(ci == 0 and p == 0),
                            stop=(ci == N_CH - 1 and p == P - 1),
                            tile_position=(0, sslot * 32),
                        )
        sT = spool.tile([128, 1024], _f32)
        nc.scalar.copy(
            sT[:].rearrange("r (g b) -> r g b", g=8, b=B),
            psB[:].rearrange("r (g w) -> r g w", g=8, w=512)[:, :, :B])
        nc.sync.dma_start(sp[:], sT[:])
    return nc


# ---------------------------------------------------------------------------
# Host glue
def _squash(s):
    v = s.reshape(B, J, D)
    s2 = np.sum(np.square(v), axis=-1, keepdims=True)
    scale = s2 / (1.0 + s2) / np.sqrt(s2 + EPS)
    return (scale * v).astype(np.float32)


_cache = {}


def _get_nc(name):
    if name not in _cache:
        _cache[name] = build_l1() if name == "l1" else build_l2()
    return _cache[name]


def _prep_inputs(x, W):
    """Per-core host-side re-layouts (bf16)."""
    per_core = []
    for c in range(N_CORES):
        sl = slice(c * I_LOC, (c + 1) * I_LOC)
        xc = x[:, sl, :]                                   # [B, I_LOC, P]
        wc = W[:, sl, :, :]                                # [J, I_LOC, D, P]
        # L1: interleave x/W per 128-row chunk of (i,p)
        xp = np.ascontiguousarray(
            xc.transpose(1, 2, 0).reshape(I_LOC * P, B))   # (i,p),b
        wt = np.ascontiguousarray(
            wc.transpose(1, 3, 0, 2).reshape(I_LOC * P, JD))  # (i,p),(j,d)
        n_chunks = (I_LOC * P) // 128
        xw1 = np.empty((n_chunks, 128, B + JD), _npbf)
        xw1[:, :, :B] = xp.reshape(n_chunks, 128, B).astype(_npbf)
        xw1[:, :, B:] = wt.reshape(n_chunks, 128, JD).astype(_npbf)
        # WA[slot*32+d, grp*2048 + i*8 + p] = wc[grp*4+slot, i, d, p]
        wa4 = np.zeros((4, 32, 8, I_LOC * P), _npbf)
        wa4[:, :D] = wc.reshape(8, 4, I_LOC, D * P).transpose(
            1, 0, 2, 3).reshape(4, 8, I_LOC, D, P).transpose(
            0, 3, 1, 2, 4).reshape(4, D, 8, I_LOC * P).astype(_npbf)
        wa = wa4.reshape(128, 8 * I_LOC * P)
        # xa[b, i*8+p]
        xa = np.ascontiguousarray(xc.reshape(B, I_LOC * P)).astype(_npbf)
        # xbt[ilo, ihi*1024 + p*128 + b] = xc[b, ihi*128+ilo, p]
        xbt_ = np.ascontiguousarray(
            xc.reshape(B, N_CH, 128, P).transpose(2, 1, 3, 0)
        ).reshape(128, N_CH * P * B).astype(_npbf)
        # wb[ilo, j*256 + ihi*128 + p*16 + d] = wc[j, ihi*128+ilo, d, p]
        wb_ = np.ascontiguousarray(
            wc.reshape(J, N_CH, 128, D, P).transpose(2, 0, 1, 4, 3)
        ).reshape(128, J * N_CH * P * D).astype(_npbf)
        per_core.append({"xw1": xw1, "wa": wa, "xa": xa,
                         "xbt": xbt_, "wb": wb_})
    return per_core


def _oa_layout(O):
    """O [B, JD] f32 -> OA[slot*32+d, grp*128+b] bf16 (zero-padded rows)."""
    o4 = np.zeros((4, 32, 8, B), _npbf)
    o4[:, :D] = O.reshape(B, 8, 4, D).transpose(2, 3, 1, 0).astype(_npbf)
    return o4.reshape(128, 8 * B)


def _sT_to_s(sT):
    """sT [128, 1024] f32 -> s [B, JD] f32; s[b, j, d] at
    sT[(j%4)*32 + d, (j//4)*128 + b]."""
    t = sT.reshape(4, 32, 8, 128)[:, :D]          # [slot, d, grp, b]
    t = t.transpose(3, 2, 0, 1)                   # [b, grp, slot, d]
    return np.ascontiguousarray(t).reshape(B, JD)


def _run(nc, in_maps, **kw):
    return run_bass_kernel_spmd(nc, in_maps, list(range(N_CORES)), **kw)


def kernel(x, W, _collect_times=None):
    x = np.asarray(x, dtype=np.float32)
    W = np.asarray(W, dtype=np.float32)
    pc = _prep_inputs(x, W)

    nc1 = _get_nc("l1")
    nc2 = _get_nc("l2")

    r1 = _run(nc1, [{"xw1": p["xw1"]} for p in pc])
    s0 = np.sum([np.asarray(r1.results[c]["sp"], np.float32)
                 for c in range(N_CORES)], axis=0)
    s0 *= (1.0 / J)
    out0 = _squash(s0)
    O1 = out0.reshape(B, JD)

    oa1 = _oa_layout(O1)
    r2 = _run(nc2, [
        {"wa": p["wa"], "oa": oa1, "xa": p["xa"], "xbt": p["xbt"],
         "wb": p["wb"]} for p in pc
    ])
    s1 = np.sum([_sT_to_s(np.asarray(r2.results[c]["sp"], np.float32))
                 for c in range(N_CORES)], axis=0)
    out1 = _squash(s1)
    O2 = (out0 + out1).reshape(B, JD)

    oa2 = _oa_layout(O2)
    r3 = _run(nc2, [
        {"wa": p["wa"], "oa": oa2, "xa": p["xa"], "xbt": p["xbt"],
         "wb": p["wb"]} for p in pc
    ])
    s2 = np.sum([_sT_to_s(np.asarray(r3.results[c]["sp"], np.float32))
                 for c in range(N_CORES)], axis=0)
    out2 = _squash(s2)

    if _collect_times is not None:
        for r in (r1, r2, r3):
            _collect_times.append(r.exec_time_ns)
    return out2


# revision 34
# speedup vs baseline: 3.5061x; 1.4501x over previous
"""CapsuleLayer dynamic-routing kernel for 8 Trainium2 NeuronCores (v2).

Problem: x [128, 2048, 8], W [32, 2048, 16, 8] (fp32)
  u_hat[b,j,i,d] = sum_p W[j,i,d,p] * x[b,i,p]
  3 rounds of routing (softmax over j, squash). Logits never materialize:
  round-k coupling g = O_k . u_hat with O_k = sum of previous outputs.

Sharding: i (input capsules) split 8 ways (I_LOC=256/core); every core
holds the full batch. Host reduces the per-core partial sums s[b,j,d]
and applies squash between the three launches.

All device data is bf16 (tolerance 2e-2; bf16 lands ~1e-3):
 - fp32 matmuls cost 4 cyc/row on the PE vs 1 for bf16,
 - DVE gets the 2x perf mode only for 2-byte dtypes.

Round structure per launch (L2 = L3), stage-dedicated engines so each
queue is fed feed-forward (the tile scheduler is static; mixed-role
queues head-of-line block):
  phase A:  V_j[b,(i,p)] = O_j @ W_j           (PE, psum, 32-row j-pair
                                                slots for quadrant rules)
            Vb_j = bf16 copy of psum           (Act; last 2 j's skip it)
            xV_j = x * Vb_j; tree lvl1         (DVE, 2x bf16)
            g_j[b,i] = tree lvl2+lvl3          (Pool)
            e = exp(g) quads + eT transposes   (Act + DMA xbar, pushed to
                                                the schedule tail via
                                                tile_wait_until)
  softmax:  Z via per-8j partial trees (DVE, overlapped), Zr = 1/Z,
            ZrT via PE transpose, x'T = xT * ZrT
  phase B:  xcT[(i),(j,p,b)] = eT * x'T        (DVE 2x; 1.5 slices on Pool)
            sT[16r,(bank,b)] += W_B^T @ xcT    (PE, 512 accumulating
                                                matmuls, one psum region
                                                per (partition-group, bank)
                                                since start_tensor_calc
                                                zeroes the whole bank row)
"""

import numpy as np
import ml_dtypes
from contextlib import ExitStack

import concourse.bass as bass
import concourse.mybir as mybir
from concourse import tile
from concourse.bass_utils import run_bass_kernel_spmd

# ---------------------------------------------------------------------------
B, I, P = 128, 2048, 8
J, D = 32, 16
JD = J * D               # 512
N_CORES = 8
I_LOC = I // N_CORES     # 256
N_CH = I_LOC // 128      # 2 partition chunks of i
EPS = 1e-7

_f32 = mybir.dt.float32
_bf16 = mybir.dt.bfloat16
_npbf = ml_dtypes.bfloat16

# engine split tuning.  Stages are engine-dedicated so every engine's
# instruction queue is fed strictly feed-forward (no head-of-line stalls):
#   PE: V matmuls | Act: psum->bf16 copies + exp | DVE: mult + tree lvl1
#   Pool: tree lvl2+lvl3 | DMA: e transposes
ACT_COPY = set(range(30))         # psum->bf16 copy on Act (else DVE
                                  # multiplies straight out of psum; the
                                  # last two j's skip the copy so the Act
                                  # exp tail starts earlier)
EXP_LAG = 32                      # j-lag before exp+transpose are issued
POOL_XC = (3, 1)                  # (jg, ci) xc slice computed on Pool


# ---------------------------------------------------------------------------
# Walrus compat: this toolchain rejects sync waits on InstDrain and >2 on
# InstEventSemaphore. Emit the waits as standalone nops before the drain.
def _apply_tile_compat():
    from concourse.vector_clock import ScopedClock

    def _strip_waits(inst):
        si = inst.sync_info
        if not si or not si.on_wait:
            return []
        waits = list(si.on_wait)
        si.on_wait = []
        inst.sync_info = si
        return waits

    def _nop_with_wait(eng, w):
        nop = eng.nop(nofuse=True, hint="drain_wait_split")
        nsi = nop.ins.sync_info
        if nsi is None:
            nsi = mybir.SyncInfo(on_wait=[], on_update=[])
        nsi.on_wait = list(nsi.on_wait or []) + [w]
        nop.ins.sync_info = nsi

    def _patched_multi_engine_barrier(self, engines):
        for inst in bass._bass_rust._multi_engine_barrier_insts(
            self, list(engines)
        ):
            eng = self.engines[inst.engine]
            for w in _strip_waits(inst):
                _nop_with_wait(eng, w)
            eng.add_instruction(inst)

    def _patched_drain_and_barrier(self, tick_clock, wait_clock):
        nop_inst = self.nc.sync.nop(nofuse=True, hint="drain_wait_split")
        wait_clock.add_sem_waits(
            nop_inst.ins, ScopedClock({None: tick_clock.global_clock})
        )
        si = nop_inst.ins.sync_info
        if si and si.on_wait and len(si.on_wait) > 1:
            extra = list(si.on_wait[1:])
            si.on_wait = [si.on_wait[0]]
            nop_inst.ins.sync_info = si
            for w in extra:
                _nop_with_wait(self.nc.sync, w)
        self.nc.sync.drain()

        self.nc.all_engine_barrier()
        assert self.sems is not None
        popped = self.nc._tile_sem_poison_stack.pop()
        assert popped is self._sem_poison
        self.nc.clear_and_free_semaphores(list(self.sems.allocated().values()))

    _WAIT_CAPS = {"InstDrain": 0, "InstEventSemaphore": 2}
    _orig_add_instruction = tile.TileContext._add_instruction

    def _patched_add_instruction(self, inst):
        si = inst.sync_info
        cap = _WAIT_CAPS.get(type(inst).__name__, 1)
        if si and si.on_wait and len(si.on_wait) > cap:
            waits = list(si.on_wait)
            si.on_wait = waits[:cap]
            inst.sync_info = si
            for w in waits[cap:]:
                nop = mybir.InstNoOp(
                    name=f"I-{self.nc.next_id()}-waitspill", ins=[], outs=[]
                )
                nop.engine = inst.engine
                nop.sync_info = mybir.SyncInfo(on_wait=[w], on_update=[])
                _orig_add_instruction(self, nop)
        _orig_add_instruction(self, inst)

    bass.Bass.multi_engine_barrier = _patched_multi_engine_barrier
    tile.TileContext._drain_and_barrier = _patched_drain_and_barrier
    tile.TileContext._add_instruction = _patched_add_instruction


_apply_tile_compat()

DEBUG_DUMPS = False

_MUL = mybir.AluOpType.mult
_ADD = mybir.AluOpType.add
_AX = mybir.AxisListType.X


# ---------------------------------------------------------------------------
# Launch 1: s0_part[b,(j,d)] = sum_{i local} u_hat[b,j,i,d]
# (iteration 0 has exactly uniform c = 1/32, applied on the host)
def build_l1():
    nc = bass.Bass("TRN2", target_bir_lowering=False, debug=False)
    n_chunks = (I_LOC * P) // 128  # 16
    CW = B + JD                    # 640 cols per chunk
    xw1 = nc.dram_tensor(
        "xw1", [n_chunks, 128, CW], _bf16, kind="ExternalInput").ap()
    sp = nc.dram_tensor("sp", [B, JD], _f32, kind="ExternalOutput").ap()
    with ExitStack() as ctx:
        tc = ctx.enter_context(tile.TileContext(nc))
        xpool = ctx.enter_context(tc.tile_pool(name="xw1", bufs=1))
        ppool = ctx.enter_context(tc.tile_pool(name="ps", bufs=1, space="PSUM"))
        opool = ctx.enter_context(tc.tile_pool(name="o", bufs=1))
        t = xpool.tile([128, n_chunks * CW], _bf16)
        tv = t[:].rearrange("r (q c) -> r q c", q=n_chunks, c=CW)
        for g in range(8):
            nc.sync.dma_start(
                tv[:, g * 2:(g + 1) * 2, :],
                xw1[g * 2:(g + 1) * 2].rearrange("q r c -> r q c"))
        psum = ppool.tile([B, JD], _f32)
        for q in range(n_chunks):
            nc.tensor.matmul(
                psum[:], lhsT=t[:, q * CW:q * CW + B],
                rhs=t[:, q * CW + B:(q + 1) * CW],
                start=(q == 0), stop=(q == n_chunks - 1),
            )
        out = opool.tile([B, JD], _f32)
        nc.scalar.copy(out[:], psum[:])
        nc.sync.dma_start(sp[:], out[:])
    return nc


# ---------------------------------------------------------------------------
# Launches 2 & 3: one routing round.
def build_l2():
    nc = bass.Bass("TRN2", target_bir_lowering=False, debug=False)
    wa = nc.dram_tensor("wa", [4, 128, I_LOC * P], _bf16,
                        kind="ExternalInput").ap()
    oa = nc.dram_tensor("oa", [128, 8 * B], _bf16, kind="ExternalInput").ap()
    xa = nc.dram_tensor("xa", [B, I_LOC * P], _bf16,
                        kind="ExternalInput").ap()
    xbt = nc.dram_tensor("xbt", [128, N_CH * P * B], _bf16,
                         kind="ExternalInput").ap()
    wb = nc.dram_tensor("wb", [128, J * N_CH * P * D], _bf16,
                        kind="ExternalInput").ap()
    ident = nc.dram_tensor("ident", [128, 128], _bf16,
                           kind="ExternalInput").ap()
    sp = nc.dram_tensor("sp", [128, 1024], _f32, kind="ExternalOutput").ap()
    if DEBUG_DUMPS:
        dbg_e = nc.dram_tensor("dbg_e", [B, J * I_LOC], _bf16,
                               kind="ExternalOutput").ap()
        dbg_eT = nc.dram_tensor("dbg_eT", [128, J * N_CH * B], _bf16,
                                kind="ExternalOutput").ap()
        dbg_xpt = nc.dram_tensor("dbg_xpt", [128, N_CH * P * B], _bf16,
                                 kind="ExternalOutput").ap()
        dbg_z = nc.dram_tensor("dbg_z", [B, I_LOC], _f32,
                               kind="ExternalOutput").ap()

    IP = I_LOC * P  # 2048

    with ExitStack() as ctx:
        tc = ctx.enter_context(tile.TileContext(nc))
        # persistent inputs
        cpool = ctx.enter_context(tc.tile_pool(name="cst", bufs=1))
        t_oa = cpool.tile([128, 8 * B], _bf16)
        nc.sync.dma_start(t_oa[:], oa[:])
        t_wa = cpool.tile([128, 4 * IP], _bf16)
        nc.sync.dma_start(t_wa[:, :512], wa[0][:, :512])
        nc.sync.dma_start(t_wa[:, 512:IP], wa[0][:, 512:])
        t_xa = cpool.tile([B, IP], _bf16)
        nc.sync.dma_start(t_xa[:], xa[:])
        for grp in range(1, 4):
            nc.sync.dma_start(
                t_wa[:, grp * IP:(grp + 1) * IP], wa[grp])
        t_xbt = cpool.tile([128, N_CH * P * B], _bf16)
        nc.sync.dma_start(t_xbt[:], xbt[:])
        t_id = cpool.tile([128, 128], _bf16)
        nc.sync.dma_start(t_id[:], ident[:])
        t_wb = cpool.tile([128, J * N_CH * P * D], _bf16)
        nc.sync.dma_start(t_wb[:], wb[:])

        # e (= g, exp'd in place) and its transpose
        bpool = ctx.enter_context(tc.tile_pool(name="big", bufs=1))
        e_all = bpool.tile([B, J * I_LOC], _bf16)
        eT = bpool.tile([128, J * N_CH * B], _bf16)
        spool = ctx.enter_context(tc.tile_pool(name="sm", bufs=1))
        zb = spool.tile([B, 4 * I_LOC], _bf16)   # per-8j-block partial Z

        # ---- phase A ------------------------------------------------------
        with tc.tile_pool(name="vps", bufs=2, space="PSUM") as vppool, \
             tc.tile_pool(name="vb", bufs=6) as vbpool, \
             tc.tile_pool(name="xv", bufs=6) as xvpool, \
             tc.tile_pool(name="t4", bufs=6) as t4pool, \
             tc.tile_pool(name="t2", bufs=6) as t2pool:
            def emit_exp_quad(a):
                # exp for j's [a, a+4), lagged so Act never stalls on it
                sl = e_all[:, a * I_LOC:(a + 4) * I_LOC]
                nc.scalar.activation(
                    sl, sl, mybir.ActivationFunctionType.Exp)
                # eT[p=ilo, blk=(jloc, ihi), b] = e[b, blk*128 + ilo]
                dst = eT[:, a * N_CH * B:(a + 4) * N_CH * B]
                nc.sync.dma_start_transpose(
                    dst.rearrange("p (g b) -> p g b", g=4 * N_CH, b=B), sl)

            def emit_block_z(q):
                # partial Z over the 8-j block q (overlaps phase A)
                blk = e_all[:, q * 2048:(q + 1) * 2048]
                zt1b = t4pool.tile([B, I_LOC * 4], _bf16)
                nc.vector.tensor_tensor(
                    zt1b[:], blk[:, :1024], blk[:, 1024:], op=_ADD)
                zt2b = t2pool.tile([B, I_LOC * 2], _bf16)
                nc.vector.tensor_tensor(
                    zt2b[:], zt1b[:, :512], zt1b[:, 512:], op=_ADD)
                nc.vector.tensor_tensor(
                    zb[:, q * I_LOC:(q + 1) * I_LOC],
                    zt2b[:, :256], zt2b[:, 256:], op=_ADD)

            for j in range(J):
                # j = grp*8 + s*2 + h: two j's share a 32-row slot; the
                # lhsT zero-half selects which one, so no pad rows exist
                grp, r = divmod(j, 8)
                s, h = divmod(r, 2)
                r0 = s * 32
                lT = t_oa[r0:r0 + 32,
                          grp * 2 * B + h * B:grp * 2 * B + (h + 1) * B]
                vps = vppool.tile([B, IP], _f32)
                for q in range(IP // 512):
                    nc.tensor.matmul(
                        vps[:, q * 512:(q + 1) * 512],
                        lhsT=lT,
                        rhs=t_wa[r0:r0 + 32,
                                 grp * IP + q * 512:grp * IP + (q + 1) * 512],
                        start=True, stop=True, tile_position=(r0, 0),
                    )
                xv = xvpool.tile([B, IP], _bf16)
                if j in ACT_COPY:
                    vb = vbpool.tile([B, IP], _bf16)
                    nc.scalar.copy(vb[:], vps[:])
                    nc.vector.tensor_tensor(xv[:], t_xa[:], vb[:], op=_MUL)
                else:
                    nc.vector.tensor_tensor(xv[:], t_xa[:], vps[:], op=_MUL)
                # p-reduction tree: lvl1 on DVE (same engine as the mult,
                # so DVE never head-of-line blocks), lvl2+3 on Pool
                xvv = xv[:].rearrange("b (i p) -> b i p", i=I_LOC, p=P)
                t4 = t4pool.tile([B, I_LOC * 4], _bf16)
                t4v = t4[:].rearrange("b (i p) -> b i p", i=I_LOC, p=4)
                nc.vector.tensor_tensor(t4v, xvv[:, :, 0:4], xvv[:, :, 4:8],
                                        op=_ADD)
                t2 = t2pool.tile([B, I_LOC * 2], _bf16)
                t2v = t2[:].rearrange("b (i p) -> b i p", i=I_LOC, p=2)
                nc.gpsimd.tensor_tensor(t2v, t4v[:, :, 0:2], t4v[:, :, 2:4],
                                        op=_ADD)
                gv = e_all[:, j * I_LOC:(j + 1) * I_LOC].rearrange(
                    "b (i u) -> b i u", i=I_LOC, u=1)
                nc.gpsimd.tensor_tensor(gv, t2v[:, :, 0:1], t2v[:, :, 1:2],
                                        op=_ADD)
            # exps/transposes/Z-partials go at the very end of the static
            # schedule (tile's build-time scheduler would otherwise slot
            # them mid-stream where their tree deps stall the Act queue)
            for a in range(0, J, 4):
                with tc.tile_wait_until(0.5 + a * 0.002):
                    emit_exp_quad(a)
                    if a >= 4 and (a - 4) % 8 == 0:
                        emit_block_z((a - 4) // 8)

        # ---- softmax normalizer (final combine of block partials) --------
        zt4 = spool.tile([B, 512], _bf16)
        nc.vector.tensor_tensor(zt4[:], zb[:, :512], zb[:, 512:], op=_ADD)
        z = spool.tile([B, I_LOC], _f32)
        nc.vector.tensor_tensor(z[:], zt4[:, :256], zt4[:, 256:], op=_ADD)
        zr = spool.tile([B, I_LOC], _f32)
        nc.vector.reciprocal(zr[:], z[:])
        zrb = spool.tile([B, I_LOC], _bf16)
        nc.vector.tensor_copy(zrb[:], zr[:])
        # transpose Zr on the (idle) PE: ~1us faster than a DMA transpose
        zrT = spool.tile([128, N_CH * B], _bf16)
        with tc.tile_pool(name="zps", bufs=1, space="PSUM") as zpool:
            zps = zpool.tile([128, N_CH * B], _bf16)
            for cc in range(N_CH):
                nc.tensor.transpose(
                    zps[:, cc * B:(cc + 1) * B],
                    zrb[:, cc * 128:(cc + 1) * 128], t_id[:])
            nc.vector.tensor_copy(zrT[:], zps[:])
        # x'T[(ilo), (ihi, p, b)] = xT * (1/Z) broadcast over p
        xpt = spool.tile([128, N_CH * P * B], _bf16)
        nc.vector.tensor_tensor(
            xpt[:].rearrange("i (c p b) -> i c p b", c=N_CH, p=P, b=B),
            t_xbt[:].rearrange("i (c p b) -> i c p b", c=N_CH, p=P, b=B),
            zrT[:].rearrange("i (c b) -> i c b", c=N_CH, b=B)
                .unsqueeze(2).broadcast_to([128, N_CH, P, B]),
            op=_MUL)

        if DEBUG_DUMPS:
            nc.sync.dma_start(dbg_e[:], e_all[:])
            nc.sync.dma_start(dbg_eT[:], eT[:])
            nc.sync.dma_start(dbg_xpt[:], xpt[:])
            nc.sync.dma_start(dbg_z[:], z[:])

        # ---- phase B ------------------------------------------------------
        eTv = eT[:].rearrange("i (j c b) -> i j c b", j=J, c=N_CH, b=B)
        xptv = xpt[:].rearrange("i (c p b) -> i c p b", c=N_CH, p=P, b=B)
        ppool = ctx.enter_context(tc.tile_pool(name="psB", bufs=1,
                                               space="PSUM"))
        xcpool = ctx.enter_context(tc.tile_pool(name="xc", bufs=3))
        # one accumulation region per (partition-group x psum bank):
        # region j = rows (j%4)*32..+16, cols (j//4)*512..+128, so no two
        # regions share a (partition, bank) pair (start_tensor_calc zeroes
        # the full 2KB bank row on written partitions).
        psB = ppool.tile([128, 4096], _f32)
        # zero the copied region up front so the final full-width copy
        # reads defined data in the rows the matmuls never write
        nc.scalar.memzero(
            psB[:].rearrange("r (g w) -> r g w", g=8, w=512)[:, :, :B])
        xc_pool_tile = None
        xcv_21 = None
        if POOL_XC is not None:
            # Pool computes the second half of slice (2,1) plus all of
            # (3,1): ~24us of its idle phase-B time, shed from the DVE
            xcb21 = xcpool.tile([128, 8 * P * B], _bf16)
            xcv_21 = xcb21[:].rearrange("i (j p b) -> i j p b",
                                        j=8, p=P, b=B)
            nc.gpsimd.tensor_tensor(
                xcv_21[:, 4:8],
                eTv[:, 2 * 8 + 4:3 * 8, 1, :]
                    .unsqueeze(2).broadcast_to([128, 4, P, B]),
                xptv[:, 1].unsqueeze(1).broadcast_to([128, 4, P, B]),
                op=_MUL)
            jg, ci = POOL_XC
            xc_pool_tile = cpool.tile([128, 8 * P * B], _bf16)
            xcv0 = xc_pool_tile[:].rearrange(
                "i (j p b) -> i j p b", j=8, p=P, b=B)
            nc.gpsimd.tensor_tensor(
                xcv0,
                eTv[:, jg * 8:(jg + 1) * 8, ci, :]
                    .unsqueeze(2).broadcast_to([128, 8, P, B]),
                xptv[:, ci].unsqueeze(1).broadcast_to([128, 8, P, B]),
                op=_MUL)
        for jg in range(4):
            for ci in range(N_CH):
                if (jg, ci) == POOL_XC:
                    xcv = xcv0
                elif (jg, ci) == (2, 1) and xcv_21 is not None:
                    xcv = xcv_21
                    nc.vector.tensor_tensor(
                        xcv[:, 0:4],
                        eTv[:, 2 * 8:2 * 8 + 4, 1, :]
                            .unsqueeze(2).broadcast_to([128, 4, P, B]),
                        xptv[:, 1].unsqueeze(1).broadcast_to([128, 4, P, B]),
                        op=_MUL)
                else:
                    xcb = xcpool.tile([128, 8 * P * B], _bf16)
                    xcv = xcb[:].rearrange(
                        "i (j p b) -> i j p b", j=8, p=P, b=B)
                    halves = 2 if (jg, ci) == (3, 0) else 1
                    hs = 8 // halves
                    for hh in range(halves):
                        nc.vector.tensor_tensor(
                            xcv[:, hh * hs:(hh + 1) * hs],
                            eTv[:, jg * 8 + hh * hs:jg * 8 + (hh + 1) * hs,
                                ci, :].unsqueeze(2).broadcast_to(
                                    [128, hs, P, B]),
                            xptv[:, ci].unsqueeze(1).broadcast_to(
                                [128, hs, P, B]),
                            op=_MUL)
                for j8 in range(8):
                    j = jg * 8 + j8
                    sgrp, sslot = divmod(j, 4)
                    nc0 = j * (N_CH * P * D) + ci * (P * D)
                    for p in range(P):
                        nc.tensor.matmul(
                            psB[sslot * 32:sslot * 32 + 16,
                                sgrp * 512:sgrp * 512 + B],
                            lhsT=t_wb[:, nc0 + p * D:nc0 + (p + 1) * D],
                            rhs=xcv[:, j8, p, :],
                            start=(ci == 0 and p == 0),
                            stop=(ci == N_CH - 1 and p == P - 1),
                            tile_position=(0, sslot * 32),
                        )
        sT = spool.tile([128, 1024], _f32)
        nc.scalar.copy(
            sT[:, :768].rearrange("r (g b) -> r g b", g=6, b=B),
            psB[:, :6 * 512].rearrange("r (g w) -> r g w", g=6,
                                       w=512)[:, :, :B])
        nc.sync.dma_start(sp[:, :768], sT[:, :768])
        nc.scalar.copy(
            sT[:, 768:].rearrange("r (g b) -> r g b", g=2, b=B),
            psB[:, 6 * 512:].rearrange("r (g w) -> r g w", g=2,
                                       w=512)[:, :, :B])
        nc.sync.dma_start(sp[:, 768:], sT[:, 768:])
    return nc


# ---------------------------------------------------------------------------
# Host glue
def _squash(s):
    v = s.reshape(B, J, D)
    s2 = np.sum(np.square(v), axis=-1, keepdims=True)
    scale = s2 / (1.0 + s2) / np.sqrt(s2 + EPS)
    return (scale * v).astype(np.float32)


_cache = {}


def _get_nc(name):
    if name not in _cache:
        _cache[name] = build_l1() if name == "l1" else build_l2()
    return _cache[name]


def _prep_inputs(x, W):
    """Per-core host-side re-layouts (bf16)."""
    per_core = []
    for c in range(N_CORES):
        sl = slice(c * I_LOC, (c + 1) * I_LOC)
        xc = x[:, sl, :]                                   # [B, I_LOC, P]
        wc = W[:, sl, :, :]                                # [J, I_LOC, D, P]
        # L1: interleave x/W per 128-row chunk of (i,p)
        xp = np.ascontiguousarray(
            xc.transpose(1, 2, 0).reshape(I_LOC * P, B))   # (i,p),b
        wt = np.ascontiguousarray(
            wc.transpose(1, 3, 0, 2).reshape(I_LOC * P, JD))  # (i,p),(j,d)
        n_chunks = (I_LOC * P) // 128
        xw1 = np.empty((n_chunks, 128, B + JD), _npbf)
        xw1[:, :, :B] = xp.reshape(n_chunks, 128, B).astype(_npbf)
        xw1[:, :, B:] = wt.reshape(n_chunks, 128, JD).astype(_npbf)
        # wa[grp, s*32+h*16+d, i*8+p] = wc[grp*8+s*2+h, i, d, p]
        wa = np.ascontiguousarray(
            wc.reshape(4, 4, 2, I_LOC, D, P).transpose(0, 1, 2, 4, 3, 5)
        ).reshape(4, 128, I_LOC * P).astype(_npbf)
        # xa[b, i*8+p]
        xa = np.ascontiguousarray(xc.reshape(B, I_LOC * P)).astype(_npbf)
        # xbt[ilo, ihi*1024 + p*128 + b] = xc[b, ihi*128+ilo, p]
        xbt_ = np.ascontiguousarray(
            xc.reshape(B, N_CH, 128, P).transpose(2, 1, 3, 0)
        ).reshape(128, N_CH * P * B).astype(_npbf)
        # wb[ilo, j*256 + ihi*128 + p*16 + d] = wc[j, ihi*128+ilo, d, p]
        wb_ = np.ascontiguousarray(
            wc.reshape(J, N_CH, 128, D, P).transpose(2, 0, 1, 4, 3)
        ).reshape(128, J * N_CH * P * D).astype(_npbf)
        per_core.append({"xw1": xw1, "wa": wa, "xa": xa,
                         "xbt": xbt_, "wb": wb_,
                         "ident": np.eye(128, dtype=_npbf)})
    return per_core


def _oa_layout(O):
    """O [B, JD] f32 -> OA[s*32+h*16+d, (grp, h)*128+b] bf16; the half of
    each 32-row slot not holding O_j is zero so the shared WA slot's other
    j contributes nothing."""
    T = O.reshape(B, 4, 4, 2, D).transpose(2, 4, 1, 3, 0)  # [s, d, grp, h, b]
    o5 = np.zeros((4, 2, D, 4, 2, B), _npbf)
    for h in range(2):
        o5[:, h, :, :, h, :] = T[:, :, :, h, :].astype(_npbf)
    return o5.reshape(128, 8 * B)


def _sT_to_s(sT):
    """sT [128, 1024] f32 -> s [B, JD] f32; s[b, j, d] at
    sT[(j%4)*32 + d, (j//4)*128 + b]."""
    t = sT.reshape(4, 32, 8, 128)[:, :D]          # [slot, d, grp, b]
    t = t.transpose(3, 2, 0, 1)                   # [b, grp, slot, d]
    return np.ascontiguousarray(t).reshape(B, JD)


def _run(nc, in_maps, **kw):
    return run_bass_kernel_spmd(nc, in_maps, list(range(N_CORES)), **kw)


def kernel(x, W, _collect_times=None):
    x = np.asarray(x, dtype=np.float32)
    W = np.asarray(W, dtype=np.float32)
    pc = _prep_inputs(x, W)

    nc1 = _get_nc("l1")
    nc2 = _get_nc("l2")

    r1 = _run(nc1, [{"xw1": p["xw1"]} for p in pc])
    s0 = np.sum([np.asarray(r1.results[c]["sp"], np.float32)
                 for c in range(N_CORES)], axis=0)
    s0 *= (1.0 / J)
    out0 = _squash(s0)
    O1 = out0.reshape(B, JD)

    oa1 = _oa_layout(O1)
    r2 = _run(nc2, [
        {"wa": p["wa"], "oa": oa1, "xa": p["xa"], "xbt": p["xbt"],
         "wb": p["wb"], "ident": p["ident"]} for p in pc
    ])
    s1 = np.sum([_sT_to_s(np.asarray(r2.results[c]["sp"], np.float32))
                 for c in range(N_CORES)], axis=0)
    out1 = _squash(s1)
    O2 = (out0 + out1).reshape(B, JD)

    oa2 = _oa_layout(O2)
    r3 = _run(nc2, [
        {"wa": p["wa"], "oa": oa2, "xa": p["xa"], "xbt": p["xbt"],
         "wb": p["wb"], "ident": p["ident"]} for p in pc
    ])
    s2 = np.sum([_sT_to_s(np.asarray(r3.results[c]["sp"], np.float32))
                 for c in range(N_CORES)], axis=0)
    out2 = _squash(s2)

    if _collect_times is not None:
        for r in (r1, r2, r3):
            _collect_times.append(r.exec_time_ns)
    return out2
